# revision 1
# baseline (speedup 1.0000x reference)
"""STBlock (temporal attn -> spatial attn -> ChebConv + residual, relu) on 8 trn2 cores.

Sharding: data-parallel over batch B=8, one batch element per core.
Host prep: densify Chebyshev Laplacian to L (N,N), pre-transpose Vs/L, build
block-diag projection weights so Cheb+residual projections become one PSUM
accumulation group per output tile.

Per-core dataflow:
  XN[8]   (128n, 768=(f,t)) fp32 natural x tiles         <- contiguous DMA
  score_t (24,24) = sum_f sum_ntile XNf.T @ XNf           (256 small MMs, one PSUM group)
  E_att   = softmax(Ve @ sigmoid(score_t) + be)           (tiny); E4 = I4 (x) E_att, bf16
  YF4[8]  (96=(f4,u), 1024n) bf16 via 256 PE transposes of XN f-col-blocks
  TT[6]   (128=(f,t)d, 1024n) bf16 = x_TA^T: per-fgroup MM lhsT=E4 rhs=YF4
  TN[8]   (128n, 768) bf16 = x_TA natural via 48 PE transposes of TT
  SG[8]   (128, 1024) bf16 = sigmoid(score_s), score_s = TT.T@TT (bf16 MMs)
  S_att   = softmax(Vs @ SG + bs) per n-chunk (fp32 softmax); transposed -> SAT[8] bf16
  SN[8]   fp32 = x_SA = SAT.T @ TN (bf16 MMs)
  P1 = L@SN, P2 = 2*L@P1 - SN   (fp32 MMs, lhsT = LT blocks)
  per (n-chunk, t4-block): PE-transpose SN/P1/P2/XN2 col-blocks (t-major col AP)
    -> 4 accumulated fp32 MMs with block-diag W (cheb k 0..2 + residual) -> +bias, relu
  OUT rotating (128, 1536=(g,t)) -> dram (1024, 64, 24)

SBUF tag reuse (lifetimes disjoint): xn->sat->xn2, yf4->p2, tt->sn(0..5), tn->p1, sg->lt.
"""
import numpy as np

B, N, F, T, G = 8, 1024, 32, 24, 64
D = F * T            # 768
NCH = N // 128       # 8 n-chunks
DCH = D // 128       # 6 d-tiles
GT = G * T           # 1536

_compiled = {}


def _build():
    if "nc" in _compiled:
        return _compiled["nc"]
    import concourse.mybir as mybir
    import concourse.bacc as bacc
    from concourse import tile

    FP = mybir.dt.float32
    BF = mybir.dt.bfloat16
    AF = mybir.ActivationFunctionType
    OP = mybir.AluOpType
    AX = mybir.AxisListType

    nc = bacc.Bacc("TRN2", target_bir_lowering=False, debug=False)

    x_d = nc.dram_tensor("x", (N, D), FP, kind="ExternalInput").ap()
    ident_d = nc.dram_tensor("ident", (128, 128), FP, kind="ExternalInput").ap()
    vet_d = nc.dram_tensor("vet", (T, T), FP, kind="ExternalInput").ap()
    be_d = nc.dram_tensor("be", (T, T), FP, kind="ExternalInput").ap()
    vst_d = nc.dram_tensor("vst", (N, N), BF, kind="ExternalInput").ap()
    bs_d = nc.dram_tensor("bs", (N, N), FP, kind="ExternalInput").ap()
    lt_d = nc.dram_tensor("lt", (N, N), BF, kind="ExternalInput").ap()
    wbd_d = nc.dram_tensor("wbd", (4, 128, 256), BF, kind="ExternalInput").ap()
    biasf_d = nc.dram_tensor("biasf", (128, 256), FP, kind="ExternalInput").ap()
    out_d = nc.dram_tensor("out", (N, GT), FP, kind="ExternalOutput").ap()

    with tile.TileContext(nc) as tc:
        with (
            tc.tile_pool(name="persist", bufs=1) as pp,
            tc.tile_pool(name="stream", bufs=1) as sp,
            tc.tile_pool(name="psum", bufs=2, space="PSUM") as ps,
            tc.tile_pool(name="psum1", bufs=1, space="PSUM") as ps1,
        ):
            # ---- constants ----
            ident = pp.tile([128, 128], FP, tag="ident")
            nc.sync.dma_start(ident[:], ident_d[:])
            identb = pp.tile([128, 128], BF, tag="identb")
            nc.vector.tensor_copy(identb[:], ident[:])
            vet = pp.tile([T, T], FP, tag="vet")
            nc.sync.dma_start(vet[:], vet_d[:])
            be = pp.tile([T, T], FP, tag="be")
            nc.sync.dma_start(be[:], be_d[:])
            wbd = [pp.tile([128, 256], BF, name=f"wbd{k}", tag=f"wbd{k}") for k in range(4)]
            for k in range(4):
                nc.sync.dma_start(wbd[k][:], wbd_d[k])
            biasf = pp.tile([128, 256], FP, tag="biasf")
            nc.sync.dma_start(biasf[:], biasf_d[:])

            # ---- stage 0: natural x tiles (slot group A: xn -> sat -> xn2) ----
            XN = []
            for i in range(NCH):
                t = pp.tile([128, D], FP, name=f"xnA{i}", tag=f"A{i}")
                nc.sync.dma_start(t[:], x_d[i * 128:(i + 1) * 128, :])
                XN.append(t)

            # ---- stage 1: score_t ----
            ps_t = ps1.tile([T, T], FP, tag="ps_t")
            n_mm = NCH * F
            idx = 0
            for i in range(NCH):
                for f in range(F):
                    sl = XN[i][:, f * T:(f + 1) * T]
                    nc.tensor.matmul(ps_t[:], sl, sl,
                                     start=(idx == 0), stop=(idx == n_mm - 1))
                    idx += 1
            sig_t = pp.tile([T, T], FP, tag="sig_t")
            nc.scalar.activation(sig_t[:], ps_t[:], AF.Sigmoid)

            # ---- stage 2: E_att ----
            ps_e = ps1.tile([T, T], FP, tag="ps_e")
            nc.tensor.matmul(ps_e[:], vet[:], sig_t[:], start=True, stop=True)
            epre = pp.tile([T, T], FP, tag="epre")
            nc.vector.tensor_tensor(epre[:], ps_e[:], be[:], op=OP.add)
            negmax = pp.tile([T, 1], FP, tag="negmax")
            nc.vector.reduce_max(negmax[:], epre[:], axis=AX.X, negate=True)
            eexp = pp.tile([T, T], FP, tag="eexp")
            esum = pp.tile([T, 1], FP, tag="esum")
            nc.scalar.activation(eexp[:], epre[:], AF.Exp,
                                 bias=negmax[:], accum_out=esum[:])
            einv = pp.tile([T, 1], FP, tag="einv")
            nc.vector.reciprocal(einv[:], esum[:])
            eatt = pp.tile([T, T], FP, tag="eatt")
            nc.vector.tensor_scalar_mul(eatt[:], eexp[:], einv[:])
            # E4 = blockdiag(E_att x4) bf16
            e4 = pp.tile([128, 96], BF, tag="e4")
            nc.gpsimd.memset(e4[:], 0.0)
            for j in range(4):
                nc.vector.tensor_copy(e4[32 * j:32 * j + 24, 24 * j:24 * j + 24], eatt[:])

            # ---- stage 3: YF4 groups (96=(f4,u), 1024) bf16 (slot group B: yf4 -> p2) ----
            YF4 = [pp.tile([128, N], BF, name=f"yfB{g}", tag=f"B{g}") for g in range(NCH)]
            for g in range(NCH):
                nc.gpsimd.memset(YF4[g][:], 0.0)
            for i in range(NCH):
                for f in range(F):
                    pt = ps.tile([T, 128], FP, tag="ps_tr")
                    nc.tensor.transpose(pt[:], XN[i][:, f * T:(f + 1) * T], ident[:])
                    dst = YF4[f // 4][32 * (f % 4):32 * (f % 4) + 24, i * 128:(i + 1) * 128]
                    if f % 2 == 0:
                        nc.vector.tensor_copy(dst, pt[:])
                    else:
                        nc.scalar.activation(dst, pt[:], AF.Copy)

            # ---- stage 4: TT bf16 (slot group C: tt -> sn[0:6]) ----
            TT = [pp.tile([128, N], BF, name=f"ttC{p}", tag=f"C{p}") for p in range(DCH)]

            def copy_rows(dst_tiles, g0, src, rows, width):
                # copy src (rows, width) into global partition rows [g0, g0+rows);
                # 32-row pieces: nonzero-offset partition APs must stay in one quadrant
                a = g0
                while a < g0 + rows:
                    q = a // 128
                    seg = min(g0 + rows - a, 128 - (a % 128), 32)
                    s0 = a - g0
                    nc.vector.tensor_copy(dst_tiles[q][a % 128:a % 128 + seg, :width],
                                          src[s0:s0 + seg, :width])
                    a += seg

            for g in range(NCH):
                pt = ps.tile([96, N], FP, tag="ps_big")
                for half in range(2):
                    nc.tensor.matmul(pt[:, half * 512:(half + 1) * 512],
                                     e4[:],
                                     YF4[g][:, half * 512:(half + 1) * 512],
                                     start=True, stop=True)
                copy_rows(TT, 96 * g, pt[:], 96, N)

            # ---- stage 5: TN bf16 natural x_TA (slot group D: tn -> p1) ----
            TN = [pp.tile([128, D], BF, name=f"tnD{i}", tag=f"D{i}") for i in range(NCH)]
            for p in range(DCH):
                for i in range(NCH):
                    pt = ps.tile([128, 128], BF, tag="ps_tr")
                    nc.tensor.transpose(pt[:], TT[p][:, i * 128:(i + 1) * 128], identb[:])
                    dst = TN[i][:, p * 128:(p + 1) * 128]
                    if (p * NCH + i) % 2 == 0:
                        nc.vector.tensor_copy(dst, pt[:])
                    else:
                        nc.scalar.activation(dst, pt[:], AF.Copy)

            # ---- stage 6: SG bf16 (slot group E: sg -> lt) ----
            SG = [pp.tile([128, N], BF, name=f"sgE{i}", tag=f"E{i}") for i in range(NCH)]
            for i in range(NCH):
                pt = ps.tile([128, N], FP, tag="ps_big")
                for half in range(2):
                    for p in range(DCH):
                        nc.tensor.matmul(
                            pt[:, half * 512:(half + 1) * 512],
                            TT[p][:, i * 128:(i + 1) * 128],
                            TT[p][:, half * 512:(half + 1) * 512],
                            start=(p == 0), stop=(p == DCH - 1))
                nc.scalar.activation(SG[i][:], pt[:], AF.Sigmoid)

            # ---- stage 7: S_att softmax + transpose -> SAT bf16 (reuses A slots) ----
            VST = [pp.tile([128, N], BF, name=f"vst{m}", tag=f"vst{m}") for m in range(NCH)]
            for m in range(NCH):
                nc.sync.dma_start(VST[m][:], vst_d[m * 128:(m + 1) * 128, :])
            SAT = [pp.tile([128, N], BF, name=f"satA{m}", tag=f"A{m}") for m in range(NCH)]
            for i in range(NCH):
                pt = ps.tile([128, N], FP, tag="ps_big")
                for half in range(2):
                    for m in range(NCH):
                        nc.tensor.matmul(
                            pt[:, half * 512:(half + 1) * 512],
                            VST[m][:, i * 128:(i + 1) * 128],
                            SG[m][:, half * 512:(half + 1) * 512],
                            start=(m == 0), stop=(m == NCH - 1))
                bsb = sp.tile([128, N], FP, tag="bsb", bufs=2)
                nc.sync.dma_start(bsb[:], bs_d[i * 128:(i + 1) * 128, :])
                spre = sp.tile([128, N], FP, tag="spre")
                nc.vector.tensor_tensor(spre[:], pt[:], bsb[:], op=OP.add)
                nmax = sp.tile([128, 1], FP, tag="nmax")
                nc.vector.reduce_max(nmax[:], spre[:], axis=AX.X, negate=True)
                sexp = sp.tile([128, N], FP, tag="sexp")
                ssum = sp.tile([128, 1], FP, tag="ssum")
                nc.scalar.activation(sexp[:], spre[:], AF.Exp,
                                     bias=nmax[:], accum_out=ssum[:])
                sinv = sp.tile([128, 1], FP, tag="sinv")
                nc.vector.reciprocal(sinv[:], ssum[:])
                sa = sp.tile([128, N], FP, tag="sa")
                nc.vector.tensor_scalar_mul(sa[:], sexp[:], sinv[:])
                for p in range(NCH):
                    pt2 = ps.tile([128, 128], FP, tag="ps_tr")
                    nc.tensor.transpose(pt2[:], sa[:, p * 128:(p + 1) * 128], ident[:])
                    dst = SAT[p][:, i * 128:(i + 1) * 128]
                    if (i + p) % 2 == 0:
                        nc.vector.tensor_copy(dst, pt2[:])
                    else:
                        nc.scalar.activation(dst, pt2[:], AF.Copy)

            # ---- stage 8: SN fp32 = x_SA (sn[0:6] on C slots, sn6/7 fresh) ----
            SN = []
            for i in range(NCH):
                if i < DCH:
                    t = pp.tile([128, D], BF, name=f"snC{i}", tag=f"C{i}")
                else:
                    t = pp.tile([128, D], BF, name=f"sn{i}", tag=f"sn{i}")
                SN.append(t)
            for i in range(NCH):
                pt = ps.tile([128, D], FP, tag="ps_big")
                for c0, cw in ((0, 512), (512, 256)):
                    for m in range(NCH):
                        nc.tensor.matmul(
                            pt[:, c0:c0 + cw],
                            SAT[m][:, i * 128:(i + 1) * 128],
                            TN[m][:, c0:c0 + cw],
                            start=(m == 0), stop=(m == NCH - 1))
                nc.vector.tensor_copy(SN[i][:], pt[:])

            # ---- stage 9: P1 = L@SN (D slots); P2 = 2 L@P1 - SN (B slots) ----
            LT = [pp.tile([128, N], BF, name=f"ltE{m}", tag=f"E{m}") for m in range(NCH)]
            for m in range(NCH):
                nc.sync.dma_start(LT[m][:], lt_d[m * 128:(m + 1) * 128, :])
            P1 = [pp.tile([128, D], BF, name=f"p1D{i}", tag=f"D{i}") for i in range(NCH)]
            P2 = [pp.tile([128, D], BF, name=f"p2B{i}", tag=f"B{i}") for i in range(NCH)]
            for i in range(NCH):
                pt = ps.tile([128, D], FP, tag="ps_big")
                for c0, cw in ((0, 512), (512, 256)):
                    for m in range(NCH):
                        nc.tensor.matmul(
                            pt[:, c0:c0 + cw],
                            LT[m][:, i * 128:(i + 1) * 128],
                            SN[m][:, c0:c0 + cw],
                            start=(m == 0), stop=(m == NCH - 1))
                nc.vector.tensor_copy(P1[i][:], pt[:])
            for i in range(NCH):
                pt = ps.tile([128, D], FP, tag="ps_big")
                for c0, cw in ((0, 512), (512, 256)):
                    for m in range(NCH):
                        nc.tensor.matmul(
                            pt[:, c0:c0 + cw],
                            LT[m][:, i * 128:(i + 1) * 128],
                            P1[m][:, c0:c0 + cw],
                            start=(m == 0), stop=(m == NCH - 1))
                dbl = sp.tile([128, D], BF, tag="dbl")
                nc.scalar.activation(dbl[:], pt[:], AF.Copy, scale=2.0)
                nc.vector.tensor_tensor(P2[i][:], dbl[:], SN[i][:], op=OP.subtract)

            # ---- stage 10: reload x (A slots), projections + residual + relu ----
            XN2 = [pp.tile([128, D], FP, name=f"xn2A{i}", tag=f"A{i}") for i in range(NCH)]
            for i in range(NCH):
                nc.sync.dma_start(XN2[i][:], x_d[i * 128:(i + 1) * 128, :])

            for i in range(NCH):
                ob = sp.tile([128, GT], FP, tag="outbuf", bufs=2)
                srcs = (SN[i], P1[i], P2[i], XN2[i])
                # permute columns f-major -> t-major once per (chunk, tensor)
                perm = []
                for k in range(4):
                    sc = sp.tile([128, D], BF, name=f"perm{k}", tag=f"perm{k}", bufs=1)
                    s_ap = srcs[k][:].rearrange("q (f t) -> q t f", f=F, t=T)
                    d_ap = sc[:].rearrange("q (t f) -> q t f", t=T, f=F)
                    if k % 2 == 0:
                        nc.vector.tensor_copy(d_ap, s_ap)
                    else:
                        nc.scalar.activation(d_ap, s_ap, AF.Copy)
                    perm.append(sc)
                for p in range(DCH):
                    ptm = ps.tile([128, 256], FP, tag="ps_big")
                    for k in range(4):
                        ptr = ps.tile([128, 128], BF, tag="ps_tr")
                        nc.tensor.transpose(ptr[:], perm[k][:, p * 128:(p + 1) * 128], identb[:])
                        scr = sp.tile([128, 128], BF, name=f"scr{k % 2}",
                                      tag=f"scr{k % 2}", bufs=2)
                        if k % 2 == 0:
                            nc.vector.tensor_copy(scr[:], ptr[:])
                        else:
                            nc.scalar.activation(scr[:], ptr[:], AF.Copy)
                        nc.tensor.matmul(ptm[:], scr[:], wbd[k][:],
                                         start=(k == 0), stop=(k == 3))
                    acc = sp.tile([128, 256], FP, tag="acc", bufs=2)
                    nc.vector.tensor_tensor(acc[:], ptm[:], biasf[:], op=OP.add)
                    dst = ob[:].rearrange("q (g t) -> q g t", g=G, t=T)[:, :, 4 * p:4 * p + 4]
                    src = acc[:].rearrange("q (g t) -> q g t", g=G, t=4)
                    nc.scalar.activation(dst, src, AF.Relu)
                nc.sync.dma_start(out_d[i * 128:(i + 1) * 128, :], ob[:])

    nc.compile()
    _compiled["nc"] = nc
    return nc


def _host_prep(x, edge_index, edge_weight, Ve, be, Vs, bs, cheb_W, cheb_b, res_W, res_b):
    import ml_dtypes
    row = np.asarray(edge_index[0]).astype(np.int64)
    col = np.asarray(edge_index[1]).astype(np.int64)
    w = np.asarray(edge_weight, np.float64).copy()
    w[row == col] = 0.0
    deg = np.zeros(N, np.float64)
    np.add.at(deg, row, w)
    dis = np.where(deg > 0, 1.0 / np.sqrt(np.where(deg > 0, deg, 1.0)), 0.0)
    norm = -dis[row] * w * dis[col]
    L = np.zeros((N, N), np.float64)
    np.add.at(L, (col, row), norm)

    cheb_W = np.asarray(cheb_W, np.float32)
    res_W = np.asarray(res_W, np.float32)
    wbd = np.zeros((4, 128, 256), np.float32)
    for tp in range(4):
        for k in range(3):
            wbd[k, tp * 32:(tp + 1) * 32, tp::4] = cheb_W[k]          # (F, G)
        wbd[3, tp * 32:(tp + 1) * 32, tp::4] = res_W.T                # (F, G)
    bias1 = (np.asarray(cheb_b, np.float32) + np.asarray(res_b, np.float32))
    biasf = np.repeat(np.repeat(bias1.reshape(1, G, 1), 4, axis=2).reshape(1, 256),
                      128, axis=0).astype(np.float32)

    return {
        "ident": np.eye(128, dtype=np.float32),
        "vet": np.ascontiguousarray(np.asarray(Ve, np.float32).T),
        "be": np.ascontiguousarray(np.asarray(be, np.float32)[0]),
        "vst": np.ascontiguousarray(np.asarray(Vs, np.float32).T).astype(ml_dtypes.bfloat16),
        "bs": np.ascontiguousarray(np.asarray(bs, np.float32)[0]),
        "lt": np.ascontiguousarray(L.T.astype(np.float32)).astype(ml_dtypes.bfloat16),
        "wbd": wbd.astype(ml_dtypes.bfloat16),
        "biasf": biasf,
    }


TRACE = False
LAST = {}


def kernel(x, edge_index, edge_weight, Ve, be, Vs, bs, cheb_W, cheb_b, res_W, res_b):
    from concourse.bass_utils import run_bass_kernel_spmd

    x = np.asarray(x, np.float32)
    shared = _host_prep(x, edge_index, edge_weight, Ve, be, Vs, bs,
                        cheb_W, cheb_b, res_W, res_b)
    nc = _build()
    in_maps = []
    for b in range(B):
        m = dict(shared)
        m["x"] = np.ascontiguousarray(x[b].reshape(N, D))
        in_maps.append(m)
    res = run_bass_kernel_spmd(nc, in_maps, list(range(B)), trace=TRACE)
    LAST["res"] = res
    out = np.stack([r["out"].reshape(N, G, T) for r in res.results], axis=0)
    return out



# revision 4
# speedup vs baseline: 1.1335x; 1.1335x over previous
"""STBlock (temporal attn -> spatial attn -> ChebConv + residual, relu) on 8 trn2 cores.

Sharding: data-parallel over batch B=8, one batch element per core.

v2 design: t-major feature layout d' = t*32 + f everywhere, zero PE transposes.
  - host uploads x twice in bf16: natural f-major (score_t) and pre-transposed
    t-major (768, 1024) for XT.
  - temporal attention applied in transposed space: TT' = (E_att^T (x) I32) @ XT
    via an on-device Kronecker-expanded E'' (built with 6 tiny matmuls + 36
    broadcast multiplies).
  - all on-device transposes (TN, SAT, SNT, P1T) are DMA xbar block transposes
    (bf16 128x128) on otherwise-idle DMA engines.
  - softmax skips max-subtraction (logits bounded by |Vs| row sums) and defers
    normalization into the SN psum evacuation scale (per-partition 1/sum).
  - final Cheb+residual projection consumes 64-row (2t x 32f) strips of the
    t-major transposed tensors against block-diag weights: 96 K=64 matmuls,
    output produced transposed (1536, 1024) and un-transposed on host.

Per-core dataflow:
  XN[8]  (128n, 768=(t,f)) bf16   <- dma xb16  (f-major natural, score_t only)
  XT[6]  (128d', 1024n) bf16      <- dma xb16T (host-transposed t-major)
  score_t (24,24) = 256 bf16 MMs; E_att = softmax(Ve sigmoid(score_t) + be)
  E2[6]  (128, 768) bf16 = E'' chunks (Kron expand of eatt via REP-matmuls + P32)
  TT[6]  (128d', 1024n) bf16 = sum_p E2[p,q].T @ XT[p]   (72 MMs)
  TN[8]  (128n, 768d') bf16 <- 48 dma transposes of TT
  SG[8]  (128, 1024) bf16 = sigmoid(TT.T TT)             (96 MMs)
  eexp_i (128, 1024) bf16 = exp(Vs@SG + bs), row sums -> sinv[i] (128 MMs)
  SAT[8] (128m, 1024n) bf16 <- 64 dma transposes of eexp
  SN[8]  (128n, 768) bf16 = sinv * SAT.T @ TN            (128 MMs)
  SNT[6] <- 48 dma transposes; P1[8] = LT.T @ SN (128 MMs); P1T[6] <- 48 dma T
  P2T[6] = 2*(P1.T@LT per chunk) - SNT                   (96 MMs)
  OUTT chunks c=0..11 (128=(2t,64g), 1024n): 4 accumulated K=64 MMs per half
    vs block-diag W4 (SNT,P1T,P2T,XT) -> relu(+bias) -> dma (1536,1024) fp32
Host un-transposes the output.
"""
import numpy as np

B, N, F, T, G = 8, 1024, 32, 24, 64
D = F * T            # 768
NCH = N // 128       # 8 n-chunks
DCH = D // 128       # 6 d-chunks
GT = G * T           # 1536

_compiled = {}


def _build():
    if "nc" in _compiled:
        return _compiled["nc"]
    import concourse.mybir as mybir
    import concourse.bacc as bacc
    from concourse import tile

    FP = mybir.dt.float32
    BF = mybir.dt.bfloat16
    AF = mybir.ActivationFunctionType
    OP = mybir.AluOpType

    nc = bacc.Bacc("TRN2", target_bir_lowering=False, debug=False)

    xb_d = nc.dram_tensor("xb", (N, D), BF, kind="ExternalInput").ap()
    xbt_d = nc.dram_tensor("xbt", (D, N), BF, kind="ExternalInput").ap()
    vet_d = nc.dram_tensor("vet", (T, T), FP, kind="ExternalInput").ap()
    be_d = nc.dram_tensor("be", (T, T), FP, kind="ExternalInput").ap()
    rep_d = nc.dram_tensor("rep", (T, DCH * 128), FP, kind="ExternalInput").ap()
    p32_d = nc.dram_tensor("p32", (128, 128), BF, kind="ExternalInput").ap()
    vst_d = nc.dram_tensor("vst", (N, N), BF, kind="ExternalInput").ap()
    bs_d = nc.dram_tensor("bs", (N, N), FP, kind="ExternalInput").ap()
    lt_d = nc.dram_tensor("lt", (N, N), BF, kind="ExternalInput").ap()
    w4_d = nc.dram_tensor("w4", (4, 128, 128), BF, kind="ExternalInput").ap()
    biast_d = nc.dram_tensor("biast", (128, 1), FP, kind="ExternalInput").ap()
    out_d = nc.dram_tensor("out", (GT, N), FP, kind="ExternalOutput").ap()

    with tile.TileContext(nc) as tc:
        with (
            tc.tile_pool(name="persist", bufs=1) as pp,
            tc.tile_pool(name="stream", bufs=1) as sp,
            tc.tile_pool(name="psum", bufs=2, space="PSUM") as ps,
            tc.tile_pool(name="psum768", bufs=2, space="PSUM") as ps7,
            tc.tile_pool(name="psum1", bufs=1, space="PSUM") as ps1,
        ):
            def copy3(idx, dst, src):
                if idx % 2 == 0:
                    nc.vector.tensor_copy(dst, src)
                else:
                    nc.scalar.copy(dst, src)

            def vg(idx):
                return nc.vector if idx % 2 == 0 else nc.gpsimd

            # ---- constants ----
            vet = pp.tile([T, T], FP, tag="vet")
            nc.sync.dma_start(vet[:], vet_d[:])
            be = pp.tile([T, T], FP, tag="be")
            nc.sync.dma_start(be[:], be_d[:])
            rep = pp.tile([T, DCH * 128], FP, tag="rep")
            nc.sync.dma_start(rep[:], rep_d[:])
            p32 = pp.tile([128, 128], BF, tag="p32")
            nc.sync.dma_start(p32[:], p32_d[:])
            w4 = [pp.tile([128, 128], BF, name=f"w4{k}", tag=f"w4{k}") for k in range(4)]
            for k in range(4):
                nc.sync.dma_start(w4[k][:], w4_d[k])
            biast = pp.tile([128, 1], FP, tag="biast")
            nc.sync.dma_start(biast[:], biast_d[:])

            # ---- stage 0: inputs ----
            XN = []
            for i in range(NCH):
                t = pp.tile([128, D], BF, name=f"xnA{i}", tag=f"A{i}")
                nc.sync.dma_start(t[:], xb_d[i * 128:(i + 1) * 128, :])
                XN.append(t)
            XT = []
            for p in range(DCH):
                t = pp.tile([128, N], BF, name=f"xt{p}", tag=f"xt{p}")
                nc.sync.dma_start(t[:], xbt_d[p * 128:(p + 1) * 128, :])
                XT.append(t)
            VST = [pp.tile([128, N], BF, name=f"vstE{m}", tag=f"E{m}") for m in range(NCH)]
            for m in range(NCH):
                nc.sync.dma_start(VST[m][:], vst_d[m * 128:(m + 1) * 128, :])
            LT = [pp.tile([128, N], BF, name=f"lt{m}", tag=f"lt{m}") for m in range(NCH)]
            for m in range(NCH):
                nc.sync.dma_start(LT[m][:], lt_d[m * 128:(m + 1) * 128, :])

            # ---- stage 1: score_t (bf16, f-major slices of XN) ----
            ps_t = ps1.tile([T, T], FP, tag="ps_t")
            n_mm = NCH * F
            idx = 0
            for i in range(NCH):
                for f in range(F):
                    sl = XN[i][:, f * T:(f + 1) * T]
                    nc.tensor.matmul(ps_t[:], sl, sl,
                                     start=(idx == 0), stop=(idx == n_mm - 1))
                    idx += 1
            sig_t = pp.tile([T, T], FP, tag="sig_t")
            nc.scalar.activation(sig_t[:], ps_t[:], AF.Sigmoid)

            # ---- stage 2: E_att (no max-sub; logits bounded) ----
            ps_e = ps1.tile([T, T], FP, tag="ps_t")
            nc.tensor.matmul(ps_e[:], vet[:], sig_t[:], start=True, stop=True)
            epre = pp.tile([T, T], FP, tag="epre")
            nc.vector.tensor_tensor(epre[:], ps_e[:], be[:], op=OP.add)
            eexp = pp.tile([T, T], FP, tag="eexp")
            esum = pp.tile([T, 1], FP, tag="esum")
            nc.scalar.activation(eexp[:], epre[:], AF.Exp, accum_out=esum[:])
            einv = pp.tile([T, 1], FP, tag="einv")
            nc.vector.reciprocal(einv[:], esum[:])
            eatt = pp.tile([T, T], FP, tag="eatt")
            nc.vector.tensor_scalar_mul(eatt[:], eexp[:], einv[:])

            # E_EXP_p (128, 24) = REP_p.T @ eatt ; E2[p] (128, 768) Kron chunks
            EX = [pp.tile([128, T], FP, name=f"ex{p}", tag=f"ex{p}") for p in range(DCH)]
            for p in range(DCH):
                pe = ps1.tile([128, T], FP, tag="ps_ex")
                nc.tensor.matmul(pe[:], rep[:, p * 128:(p + 1) * 128], eatt[:],
                                 start=True, stop=True)
                copy3(p, EX[p][:], pe[:])
            E2 = [pp.tile([128, D], BF, name=f"e2B{p}", tag=f"B{p}") for p in range(DCH)]
            p32v = p32[:].rearrange("r (b j) -> r b j", b=4, j=32)
            for p in range(DCH):
                for q in range(DCH):
                    dst = E2[p][:, q * 128:(q + 1) * 128].rearrange(
                        "r (b j) -> r b j", b=4, j=32)
                    src = EX[p][:, 4 * q:4 * q + 4].broadcast_to((128, 4, 32))
                    vg(p * DCH + q).tensor_tensor(dst, p32v, src, op=OP.mult)

            # ---- stage 3: TT' = sum_p E2[p][:,q].T @ XT[p] ----
            TT = [pp.tile([128, N], BF, name=f"ttC{q}", tag=f"C{q}") for q in range(DCH)]
            for q in range(DCH):
                for h in range(2):
                    pt = ps.tile([128, 512], FP, tag="ps_big")
                    for p in range(DCH):
                        nc.tensor.matmul(pt[:], E2[p][:, q * 128:(q + 1) * 128],
                                         XT[p][:, h * 512:(h + 1) * 512],
                                         start=(p == 0), stop=(p == DCH - 1))
                    copy3(q * 2 + h, TT[q][:, h * 512:(h + 1) * 512], pt[:])

            # ---- stage 4: TN (natural x_TA) <- dma transposes of TT ----
            TN = []
            for i in range(NCH):
                tag = f"B{i}" if i < DCH else f"tn{i}"
                TN.append(pp.tile([128, D], BF, name=f"tn{i}", tag=tag))
            for p in range(DCH):
                for i in range(NCH):
                    nc.sync.dma_start_transpose(
                        TN[i][:, p * 128:(p + 1) * 128],
                        TT[p][:, i * 128:(i + 1) * 128])

            # ---- stage 5: SG = sigmoid(score_s) ----
            SG = [pp.tile([128, N], BF, name=f"sgD{i}", tag=f"D{i}") for i in range(NCH)]
            for i in range(NCH):
                for h in range(2):
                    pt = ps.tile([128, 512], FP, tag="ps_big")
                    for p in range(DCH):
                        nc.tensor.matmul(pt[:], TT[p][:, i * 128:(i + 1) * 128],
                                         TT[p][:, h * 512:(h + 1) * 512],
                                         start=(p == 0), stop=(p == DCH - 1))
                    nc.scalar.activation(SG[i][:, h * 512:(h + 1) * 512],
                                         pt[:], AF.Sigmoid)

            # ---- stage 6: eexp_i = exp(Vs@SG + bs); SAT via dma transpose ----
            SAT = [pp.tile([128, N], BF, name=f"satA{m}", tag=f"A{m}") for m in range(NCH)]
            SINV = [pp.tile([128, 1], FP, name=f"sinv{i}", tag=f"sinv{i}")
                    for i in range(NCH)]
            for i in range(NCH):
                spre = sp.tile([128, N], FP, tag="spre", bufs=2)
                bsb = sp.tile([128, N], FP, tag="bsb", bufs=2)
                nc.sync.dma_start(bsb[:], bs_d[i * 128:(i + 1) * 128, :])
                for h in range(2):
                    pt = ps.tile([128, 512], FP, tag="ps_big")
                    for m in range(NCH):
                        nc.tensor.matmul(pt[:], VST[m][:, i * 128:(i + 1) * 128],
                                         SG[m][:, h * 512:(h + 1) * 512],
                                         start=(m == 0), stop=(m == NCH - 1))
                    nc.vector.tensor_tensor(spre[:, h * 512:(h + 1) * 512], pt[:],
                                            bsb[:, h * 512:(h + 1) * 512], op=OP.add)
                sexp = sp.tile([128, N], BF, tag="sexp", bufs=2)
                ssum = sp.tile([128, 1], FP, tag="ssum", bufs=2)
                nc.scalar.activation(sexp[:], spre[:], AF.Exp, accum_out=ssum[:])
                nc.vector.reciprocal(SINV[i][:], ssum[:])
                for m in range(NCH):
                    nc.sync.dma_start_transpose(
                        SAT[m][:, i * 128:(i + 1) * 128],
                        sexp[:, m * 128:(m + 1) * 128])

            # ---- stage 7: SN = sinv * (SAT.T @ TN) ----
            SN = [pp.tile([128, D], BF, name=f"snF{i}", tag=f"F{i}") for i in range(NCH)]
            for i in range(NCH):
                pt = ps7.tile([128, D], FP, tag="ps_768")
                for c0, cw in ((0, 512), (512, 256)):
                    for m in range(NCH):
                        nc.tensor.matmul(pt[:, c0:c0 + cw],
                                         SAT[m][:, i * 128:(i + 1) * 128],
                                         TN[m][:, c0:c0 + cw],
                                         start=(m == 0), stop=(m == NCH - 1))
                nc.vector.tensor_scalar_mul(SN[i][:], pt[:], SINV[i][:])

            # ---- stage 8: SNT <- dma transposes of SN (reuse C slots) ----
            SNT = [pp.tile([128, N], BF, name=f"sntC{q}", tag=f"C{q}") for q in range(DCH)]
            for i in range(NCH):
                for q in range(DCH):
                    nc.sync.dma_start_transpose(
                        SNT[q][:, i * 128:(i + 1) * 128],
                        SN[i][:, q * 128:(q + 1) * 128])

            # ---- stage 9: P1 = L @ SN (natural) ----
            P1 = [pp.tile([128, D], BF, name=f"p1G{i}", tag=f"G{i}") for i in range(NCH)]
            for i in range(NCH):
                pt = ps7.tile([128, D], FP, tag="ps_768")
                for c0, cw in ((0, 512), (512, 256)):
                    for m in range(NCH):
                        nc.tensor.matmul(pt[:, c0:c0 + cw],
                                         LT[m][:, i * 128:(i + 1) * 128],
                                         SN[m][:, c0:c0 + cw],
                                         start=(m == 0), stop=(m == NCH - 1))
                copy3(i, P1[i][:], pt[:])

            # ---- stage 10: P1T <- dma transposes (reuse D slots) ----
            P1T = [pp.tile([128, N], BF, name=f"p1tD{q}", tag=f"D{q}") for q in range(DCH)]
            for i in range(NCH):
                for q in range(DCH):
                    nc.sync.dma_start_transpose(
                        P1T[q][:, i * 128:(i + 1) * 128],
                        P1[i][:, q * 128:(q + 1) * 128])

            # ---- stage 11: P2T = 2*(P1.T-chunk @ LT) - SNT (reuse E slots) ----
            P2T = [pp.tile([128, N], BF, name=f"p2tE{q}", tag=f"E{q}") for q in range(DCH)]
            for q in range(DCH):
                for h in range(2):
                    pt = ps.tile([128, 512], FP, tag="ps_big")
                    for m in range(NCH):
                        nc.tensor.matmul(pt[:], P1[m][:, q * 128:(q + 1) * 128],
                                         LT[m][:, h * 512:(h + 1) * 512],
                                         start=(m == 0), stop=(m == NCH - 1))
                    nc.vector.scalar_tensor_tensor(
                        P2T[q][:, h * 512:(h + 1) * 512], pt[:], 2.0,
                        SNT[q][:, h * 512:(h + 1) * 512],
                        op0=OP.mult, op1=OP.subtract)

            # ---- stage 12: OUTT chunks: 4 accumulated K=64 MMs + relu ----
            for c in range(12):
                p, b = c // 2, c % 2
                r0 = 64 * b
                ob = sp.tile([128, N], FP, tag="outbuf", bufs=3)
                srcs = (SNT[p], P1T[p], P2T[p], XT[p])
                for h in range(2):
                    pt = ps.tile([128, 512], FP, tag="ps_big")
                    for k in range(4):
                        nc.tensor.matmul(pt[:], w4[k][r0:r0 + 64, :],
                                         srcs[k][r0:r0 + 64, h * 512:(h + 1) * 512],
                                         start=(k == 0), stop=(k == 3))
                    dst = ob[:, h * 512:(h + 1) * 512]
                    if (c * 2 + h) % 2 == 0:
                        nc.scalar.activation(dst, pt[:], AF.Relu, bias=biast[:])
                    else:
                        nc.vector.tensor_scalar(dst, pt[:], biast[:], 0.0,
                                                op0=OP.add, op1=OP.max)
                nc.sync.dma_start(out_d[c * 128:(c + 1) * 128, :], ob[:])

    nc.compile()
    _compiled["nc"] = nc
    return nc


def _host_prep(x, edge_index, edge_weight, Ve, be, Vs, bs, cheb_W, cheb_b, res_W, res_b):
    import ml_dtypes
    BF = ml_dtypes.bfloat16
    row = np.asarray(edge_index[0]).astype(np.int64)
    col = np.asarray(edge_index[1]).astype(np.int64)
    w = np.asarray(edge_weight, np.float64).copy()
    w[row == col] = 0.0
    deg = np.zeros(N, np.float64)
    np.add.at(deg, row, w)
    dis = np.where(deg > 0, 1.0 / np.sqrt(np.where(deg > 0, deg, 1.0)), 0.0)
    norm = -dis[row] * w * dis[col]
    L = np.zeros((N, N), np.float64)
    np.add.at(L, (col, row), norm)

    cheb_W = np.asarray(cheb_W, np.float32)   # (K, F, G)
    res_W = np.asarray(res_W, np.float32)     # (G, F)
    Wk = [cheb_W[0], cheb_W[1], cheb_W[2], res_W.T]
    w4 = np.zeros((4, 128, 128), np.float32)
    for k in range(4):
        for c4 in range(4):
            c2 = c4 % 2
            w4[k, c4 * 32:(c4 + 1) * 32, c2 * 64:(c2 + 1) * 64] = Wk[k]
    b1 = (np.asarray(cheb_b, np.float32) + np.asarray(res_b, np.float32))
    biast = np.tile(b1, 2).reshape(128, 1).astype(np.float32)

    rep = np.zeros((T, DCH * 128), np.float32)
    for p in range(DCH):
        for a in range(4):
            u = 4 * p + a
            rep[u, p * 128 + 32 * a: p * 128 + 32 * a + 32] = 1.0
    p32 = np.zeros((128, 128), np.float32)
    for a in range(4):
        for b_ in range(4):
            p32[a * 32:(a + 1) * 32, b_ * 32:(b_ + 1) * 32] = np.eye(32)

    return {
        "vet": np.ascontiguousarray(np.asarray(Ve, np.float32).T),
        "be": np.ascontiguousarray(np.asarray(be, np.float32)[0]),
        "rep": rep,
        "p32": p32.astype(BF),
        "vst": np.ascontiguousarray(np.asarray(Vs, np.float32).T).astype(BF),
        "bs": np.ascontiguousarray(np.asarray(bs, np.float32)[0]),
        "lt": np.ascontiguousarray(L.T.astype(np.float32)).astype(BF),
        "w4": w4.astype(BF),
        "biast": biast,
    }


TRACE = False
LAST = {}


def kernel(x, edge_index, edge_weight, Ve, be, Vs, bs, cheb_W, cheb_b, res_W, res_b):
    import ml_dtypes
    from concourse.bass_utils import run_bass_kernel_spmd
    BF = ml_dtypes.bfloat16

    x = np.asarray(x, np.float32)
    shared = _host_prep(x, edge_index, edge_weight, Ve, be, Vs, bs,
                        cheb_W, cheb_b, res_W, res_b)
    nc = _build()
    in_maps = []
    for b in range(B):
        m = dict(shared)
        m["xb"] = np.ascontiguousarray(x[b].reshape(N, D)).astype(BF)
        # xbt: row d' = t*32+f  ->  x[b][n, f, t];  (D, N)
        m["xbt"] = np.ascontiguousarray(
            x[b].transpose(2, 1, 0).reshape(D, N)).astype(BF)
        in_maps.append(m)
    res = run_bass_kernel_spmd(nc, in_maps, list(range(B)), trace=TRACE)
    LAST["res"] = res
    # out (1536, 1024): row = c*128 + a*64 + g, t = 2c+a
    out = np.stack(
        [r["out"].reshape(12, 2, G, N).transpose(3, 2, 0, 1).reshape(N, G, T)
         for r in res.results], axis=0)
    return out


# revision 7
# speedup vs baseline: 2.0711x; 1.8271x over previous
"""STBlock (temporal attn -> spatial attn -> ChebConv + residual, relu) on 8 trn2 cores.

Sharding: data-parallel over batch B=8, one batch element per core.

v2 design: t-major feature layout d' = t*32 + f everywhere, zero PE transposes.
  - host uploads x twice in bf16: natural f-major (score_t) and pre-transposed
    t-major (768, 1024) for XT.
  - temporal attention applied in transposed space: TT' = (E_att^T (x) I32) @ XT
    via an on-device Kronecker-expanded E'' (built with 6 tiny matmuls + 36
    broadcast multiplies).
  - all on-device transposes (TN, SAT, SNT, P1T) are DMA xbar block transposes
    (bf16 128x128) on otherwise-idle DMA engines.
  - softmax skips max-subtraction (logits bounded by |Vs| row sums) and defers
    normalization into the SN psum evacuation scale (per-partition 1/sum).
  - final Cheb+residual projection consumes 64-row (2t x 32f) strips of the
    t-major transposed tensors against block-diag weights: 96 K=64 matmuls,
    output produced transposed (1536, 1024) and un-transposed on host.

Per-core dataflow:
  XN[8]  (128n, 768=(t,f)) bf16   <- dma xb16  (f-major natural, score_t only)
  XT[6]  (128d', 1024n) bf16      <- dma xb16T (host-transposed t-major)
  score_t (24,24) = 256 bf16 MMs; E_att = softmax(Ve sigmoid(score_t) + be)
  E2[6]  (128, 768) bf16 = E'' chunks (Kron expand of eatt via REP-matmuls + P32)
  TT[6]  (128d', 1024n) bf16 = sum_p E2[p,q].T @ XT[p]   (72 MMs)
  TN[8]  (128n, 768d') bf16 <- 48 dma transposes of TT
  SG[8]  (128, 1024) bf16 = sigmoid(TT.T TT)             (96 MMs)
  eexp_i (128, 1024) bf16 = exp(Vs@SG + bs), row sums -> sinv[i] (128 MMs)
  SAT[8] (128m, 1024n) bf16 <- 64 dma transposes of eexp
  SN[8]  (128n, 768) bf16 = sinv * SAT.T @ TN            (128 MMs)
  SNT[6] <- 48 dma transposes; P1[8] = LT.T @ SN (128 MMs); P1T[6] <- 48 dma T
  P2T[6] = 2*(P1.T@LT per chunk) - SNT                   (96 MMs)
  OUTT chunks c=0..11 (128=(2t,64g), 1024n): 4 accumulated K=64 MMs per half
    vs block-diag W4 (SNT,P1T,P2T,XT) -> relu(+bias) -> dma (1536,1024) fp32
Host un-transposes the output.
"""
import numpy as np

B, N, F, T, G = 8, 1024, 32, 24, 64
D = F * T            # 768
NCH = N // 128       # 8 n-chunks
DCH = D // 128       # 6 d-chunks
GT = G * T           # 1536

_compiled = {}


def _build():
    if "nc" in _compiled:
        return _compiled["nc"]
    import concourse.mybir as mybir
    import concourse.bacc as bacc
    from concourse import tile

    FP = mybir.dt.float32
    BF = mybir.dt.bfloat16
    AF = mybir.ActivationFunctionType
    OP = mybir.AluOpType

    nc = bacc.Bacc("TRN2", target_bir_lowering=False, debug=False)

    xb_d = nc.dram_tensor("xb", (N, D), BF, kind="ExternalInput").ap()
    xbt_d = nc.dram_tensor("xbt", (D, N), BF, kind="ExternalInput").ap()
    vet_d = nc.dram_tensor("vet", (T, T), FP, kind="ExternalInput").ap()
    be_d = nc.dram_tensor("be", (T, T), FP, kind="ExternalInput").ap()
    rep_d = nc.dram_tensor("rep", (T, DCH * 128), FP, kind="ExternalInput").ap()
    p32_d = nc.dram_tensor("p32", (128, 128), BF, kind="ExternalInput").ap()
    vst_d = nc.dram_tensor("vst", (N, N), BF, kind="ExternalInput").ap()
    bs_d = nc.dram_tensor("bs", (N, N), FP, kind="ExternalInput").ap()
    lt_d = nc.dram_tensor("lt", (N, N), BF, kind="ExternalInput").ap()
    w4_d = nc.dram_tensor("w4", (4, 128, 128), BF, kind="ExternalInput").ap()
    biast_d = nc.dram_tensor("biast", (128, 1), FP, kind="ExternalInput").ap()
    identb_d = nc.dram_tensor("identb", (128, 128), BF, kind="ExternalInput").ap()
    out_d = nc.dram_tensor("out", (GT, N), FP, kind="ExternalOutput").ap()

    with tile.TileContext(nc) as tc:
        with (
            tc.tile_pool(name="persist", bufs=1) as pp,
            tc.tile_pool(name="stream", bufs=1) as sp,
            tc.tile_pool(name="psum", bufs=2, space="PSUM") as ps,
            tc.tile_pool(name="psum1", bufs=1, space="PSUM") as ps1,
            tc.tile_pool(name="psumtr", bufs=2, space="PSUM") as pst,
        ):
            def copy3(idx, dst, src):
                if idx % 2 == 0:
                    nc.vector.tensor_copy(dst, src)
                else:
                    nc.scalar.copy(dst, src)

            def vg(idx):
                return nc.vector if idx % 2 == 0 else nc.gpsimd

            # ---- constants ----
            vet = pp.tile([T, T], FP, tag="vet")
            nc.sync.dma_start(vet[:], vet_d[:])
            be = pp.tile([T, T], FP, tag="be")
            nc.sync.dma_start(be[:], be_d[:])
            rep = pp.tile([T, DCH * 128], FP, tag="rep")
            nc.sync.dma_start(rep[:], rep_d[:])
            p32 = pp.tile([128, 128], BF, tag="p32")
            nc.sync.dma_start(p32[:], p32_d[:])
            w4 = [pp.tile([128, 128], BF, name=f"w4{k}", tag=f"w4{k}") for k in range(4)]
            for k in range(4):
                nc.sync.dma_start(w4[k][:], w4_d[k])
            biast = pp.tile([128, 1], FP, tag="biast")
            nc.sync.dma_start(biast[:], biast_d[:])
            identb = pp.tile([128, 128], BF, tag="identb")
            nc.sync.dma_start(identb[:], identb_d[:])

            tr_idx = [0]
            tr_tile = [None]

            def pe_transpose(dst, src):
                j = tr_idx[0] % 4
                if j == 0:
                    tr_tile[0] = pst.tile([128, 512], BF,
                                          name=f"trt{tr_idx[0]}", tag="ps_tr")
                pt = tr_tile[0][:, j * 128:(j + 1) * 128]
                nc.tensor.transpose(pt, src, identb[:])
                copy3(tr_idx[0], dst, pt)
                tr_idx[0] += 1

            # ---- stage 0: inputs ----
            XN = []
            for i in range(NCH):
                t = pp.tile([128, D], BF, name=f"xnA{i}", tag=f"A{i}")
                nc.sync.dma_start(t[:], xb_d[i * 128:(i + 1) * 128, :])
                XN.append(t)
            XT = []
            for p in range(DCH):
                t = pp.tile([128, N], BF, name=f"xt{p}", tag=f"xt{p}")
                nc.sync.dma_start(t[:], xbt_d[p * 128:(p + 1) * 128, :])
                XT.append(t)
            VST = [pp.tile([128, N], BF, name=f"vstE{m}", tag=f"E{m}") for m in range(NCH)]
            for m in range(NCH):
                nc.sync.dma_start(VST[m][:], vst_d[m * 128:(m + 1) * 128, :])
            LT = [pp.tile([128, N], BF, name=f"lt{m}", tag=f"lt{m}") for m in range(NCH)]
            for m in range(NCH):
                nc.sync.dma_start(LT[m][:], lt_d[m * 128:(m + 1) * 128, :])

            # ---- stage 1: score_t (bf16, f-major slices of XN) ----
            ps_t = ps1.tile([T, T], FP, tag="ps_t")
            n_mm = NCH * F
            idx = 0
            for i in range(NCH):
                for f in range(F):
                    sl = XN[i][:, f * T:(f + 1) * T]
                    nc.tensor.matmul(ps_t[:], sl, sl,
                                     start=(idx == 0), stop=(idx == n_mm - 1))
                    idx += 1
            sig_t = pp.tile([T, T], FP, tag="sig_t")
            nc.scalar.activation(sig_t[:], ps_t[:], AF.Sigmoid)

            # ---- stage 2: E_att (no max-sub; logits bounded) ----
            ps_e = ps1.tile([T, T], FP, tag="ps_t")
            nc.tensor.matmul(ps_e[:], vet[:], sig_t[:], start=True, stop=True)
            epre = pp.tile([T, T], FP, tag="epre")
            nc.vector.tensor_tensor(epre[:], ps_e[:], be[:], op=OP.add)
            eexp = pp.tile([T, T], FP, tag="eexp")
            esum = pp.tile([T, 1], FP, tag="esum")
            nc.scalar.activation(eexp[:], epre[:], AF.Exp, accum_out=esum[:])
            einv = pp.tile([T, 1], FP, tag="einv")
            nc.vector.reciprocal(einv[:], esum[:])
            eatt = pp.tile([T, T], FP, tag="eatt")
            nc.vector.tensor_scalar_mul(eatt[:], eexp[:], einv[:])

            # E_EXP_p (128, 24) = REP_p.T @ eatt ; E2[p] (128, 768) Kron chunks
            EX = [pp.tile([128, T], FP, name=f"ex{p}", tag=f"ex{p}") for p in range(DCH)]
            for p in range(DCH):
                pe = ps1.tile([128, T], FP, tag="ps_t")
                nc.tensor.matmul(pe[:], rep[:, p * 128:(p + 1) * 128], eatt[:],
                                 start=True, stop=True)
                copy3(p, EX[p][:], pe[:])
            E2 = [pp.tile([128, D], BF, name=f"e2B{p}", tag=f"B{p}") for p in range(DCH)]
            p32v = p32[:].rearrange("r (b j) -> r b j", b=4, j=32)
            for p in range(DCH):
                for q in range(DCH):
                    dst = E2[p][:, q * 128:(q + 1) * 128].rearrange(
                        "r (b j) -> r b j", b=4, j=32)
                    src = EX[p][:, 4 * q:4 * q + 4].broadcast_to((128, 4, 32))
                    vg(p * DCH + q).tensor_tensor(dst, p32v, src, op=OP.mult)

            # ---- stage 3: TT' = sum_p E2[p][:,q].T @ XT[p] ----
            TT = [pp.tile([128, N], BF, name=f"ttC{q}", tag=f"C{q}") for q in range(DCH)]
            for q in range(DCH):
                for h in range(2):
                    pt = ps.tile([128, 512], FP, tag="ps_big")
                    for p in range(DCH):
                        nc.tensor.matmul(pt[:], E2[p][:, q * 128:(q + 1) * 128],
                                         XT[p][:, h * 512:(h + 1) * 512],
                                         start=(p == 0), stop=(p == DCH - 1))
                    copy3(q * 2 + h, TT[q][:, h * 512:(h + 1) * 512], pt[:])

            # ---- stage 4: TN (natural x_TA) <- dma transposes of TT ----
            TN = []
            for i in range(NCH):
                tag = f"B{i}" if i < DCH else f"tn{i}"
                TN.append(pp.tile([128, D], BF, name=f"tn{i}", tag=tag))
            for p in range(DCH):
                for i in range(NCH):
                    pe_transpose(TN[i][:, p * 128:(p + 1) * 128],
                                 TT[p][:, i * 128:(i + 1) * 128])

            # ---- stage 5: SG = sigmoid(score_s) ----
            SG = [pp.tile([128, N], BF, name=f"sgD{i}", tag=f"D{i}") for i in range(NCH)]
            for i in range(NCH):
                for h in range(2):
                    pt = ps.tile([128, 512], FP, tag="ps_big")
                    for p in range(DCH):
                        nc.tensor.matmul(pt[:], TT[p][:, i * 128:(i + 1) * 128],
                                         TT[p][:, h * 512:(h + 1) * 512],
                                         start=(p == 0), stop=(p == DCH - 1))
                    nc.scalar.activation(SG[i][:, h * 512:(h + 1) * 512],
                                         pt[:], AF.Sigmoid)

            # ---- stage 6: eexp_i = exp(Vs@SG + bs); SAT via dma transpose ----
            SAT = [pp.tile([128, N], BF, name=f"satA{m}", tag=f"A{m}") for m in range(NCH)]
            SINV = [pp.tile([128, 1], FP, name=f"sinv{i}", tag=f"sinv{i}")
                    for i in range(NCH)]
            for i in range(NCH):
                spre = sp.tile([128, N], FP, tag="spre", bufs=2)
                bsb = sp.tile([128, N], FP, tag="bsb", bufs=2)
                nc.sync.dma_start(bsb[:], bs_d[i * 128:(i + 1) * 128, :])
                for h in range(2):
                    pt = ps.tile([128, 512], FP, tag="ps_big")
                    for m in range(NCH):
                        nc.tensor.matmul(pt[:], VST[m][:, i * 128:(i + 1) * 128],
                                         SG[m][:, h * 512:(h + 1) * 512],
                                         start=(m == 0), stop=(m == NCH - 1))
                    nc.vector.tensor_tensor(spre[:, h * 512:(h + 1) * 512], pt[:],
                                            bsb[:, h * 512:(h + 1) * 512], op=OP.add)
                sexp = sp.tile([128, N], BF, tag="sexp", bufs=2)
                ssum = sp.tile([128, 1], FP, tag="ssum", bufs=2)
                nc.scalar.activation(sexp[:], spre[:], AF.Exp, accum_out=ssum[:])
                nc.vector.reciprocal(SINV[i][:], ssum[:])
                for m in range(NCH):
                    pe_transpose(SAT[m][:, i * 128:(i + 1) * 128],
                                 sexp[:, m * 128:(m + 1) * 128])

            # ---- stage 7: SN = sinv * (SAT.T @ TN) ----
            SN = [pp.tile([128, D], BF, name=f"snF{i}", tag=f"F{i}") for i in range(NCH)]
            for i in range(NCH):
                pta = ps.tile([128, 512], FP, name=f"pta{i}", tag="ps_big")
                ptb = ps.tile([128, 256], FP, name=f"ptb{i}", tag="ps_med")
                for pt, c0, cw in ((pta, 0, 512), (ptb, 512, 256)):
                    for m in range(NCH):
                        nc.tensor.matmul(pt[:, :cw],
                                         SAT[m][:, i * 128:(i + 1) * 128],
                                         TN[m][:, c0:c0 + cw],
                                         start=(m == 0), stop=(m == NCH - 1))
                nc.vector.tensor_scalar_mul(SN[i][:, 0:512], pta[:], SINV[i][:])
                nc.vector.tensor_scalar_mul(SN[i][:, 512:768], ptb[:], SINV[i][:])

            # ---- stage 8: SNT <- dma transposes of SN (reuse C slots) ----
            SNT = [pp.tile([128, N], BF, name=f"sntC{q}", tag=f"C{q}") for q in range(DCH)]
            for i in range(NCH):
                for q in range(DCH):
                    pe_transpose(SNT[q][:, i * 128:(i + 1) * 128],
                                 SN[i][:, q * 128:(q + 1) * 128])

            # ---- stage 9: P1 = L @ SN (natural) ----
            P1 = [pp.tile([128, D], BF, name=f"p1G{i}", tag=f"G{i}") for i in range(NCH)]
            for i in range(NCH):
                pta = ps.tile([128, 512], FP, name=f"pta{i}", tag="ps_big")
                ptb = ps.tile([128, 256], FP, name=f"ptb{i}", tag="ps_med")
                for pt, c0, cw in ((pta, 0, 512), (ptb, 512, 256)):
                    for m in range(NCH):
                        nc.tensor.matmul(pt[:, :cw],
                                         LT[m][:, i * 128:(i + 1) * 128],
                                         SN[m][:, c0:c0 + cw],
                                         start=(m == 0), stop=(m == NCH - 1))
                copy3(i, P1[i][:, 0:512], pta[:])
                copy3(i + 1, P1[i][:, 512:768], ptb[:])

            # ---- stage 10: P1T <- dma transposes (reuse D slots) ----
            P1T = [pp.tile([128, N], BF, name=f"p1tD{q}", tag=f"D{q}") for q in range(DCH)]
            for i in range(NCH):
                for q in range(DCH):
                    pe_transpose(P1T[q][:, i * 128:(i + 1) * 128],
                                 P1[i][:, q * 128:(q + 1) * 128])

            # ---- stage 11: P2T = 2*(P1.T-chunk @ LT) - SNT (reuse E slots) ----
            P2T = [pp.tile([128, N], BF, name=f"p2tE{q}", tag=f"E{q}") for q in range(DCH)]
            for q in range(DCH):
                for h in range(2):
                    pt = ps.tile([128, 512], FP, tag="ps_big")
                    for m in range(NCH):
                        nc.tensor.matmul(pt[:], P1[m][:, q * 128:(q + 1) * 128],
                                         LT[m][:, h * 512:(h + 1) * 512],
                                         start=(m == 0), stop=(m == NCH - 1))
                    nc.vector.scalar_tensor_tensor(
                        P2T[q][:, h * 512:(h + 1) * 512], pt[:], 2.0,
                        SNT[q][:, h * 512:(h + 1) * 512],
                        op0=OP.mult, op1=OP.subtract)

            # ---- stage 12: OUTT chunks: 4 accumulated K=64 MMs + relu ----
            for c in range(12):
                p, b = c // 2, c % 2
                r0 = 64 * b
                ob = sp.tile([128, N], FP, tag="outbuf", bufs=3)
                srcs = (SNT[p], P1T[p], P2T[p], XT[p])
                for h in range(2):
                    pt = ps.tile([128, 512], FP, tag="ps_big")
                    for k in range(4):
                        nc.tensor.matmul(pt[:], w4[k][r0:r0 + 64, :],
                                         srcs[k][r0:r0 + 64, h * 512:(h + 1) * 512],
                                         start=(k == 0), stop=(k == 3))
                    dst = ob[:, h * 512:(h + 1) * 512]
                    if (c * 2 + h) % 2 == 0:
                        nc.scalar.activation(dst, pt[:], AF.Relu, bias=biast[:])
                    else:
                        nc.vector.tensor_scalar(dst, pt[:], biast[:], 0.0,
                                                op0=OP.add, op1=OP.max)
                nc.sync.dma_start(out_d[c * 128:(c + 1) * 128, :], ob[:])

    nc.compile()
    _compiled["nc"] = nc
    return nc


def _host_prep(x, edge_index, edge_weight, Ve, be, Vs, bs, cheb_W, cheb_b, res_W, res_b):
    import ml_dtypes
    BF = ml_dtypes.bfloat16
    row = np.asarray(edge_index[0]).astype(np.int64)
    col = np.asarray(edge_index[1]).astype(np.int64)
    w = np.asarray(edge_weight, np.float64).copy()
    w[row == col] = 0.0
    deg = np.zeros(N, np.float64)
    np.add.at(deg, row, w)
    dis = np.where(deg > 0, 1.0 / np.sqrt(np.where(deg > 0, deg, 1.0)), 0.0)
    norm = -dis[row] * w * dis[col]
    L = np.zeros((N, N), np.float64)
    np.add.at(L, (col, row), norm)

    cheb_W = np.asarray(cheb_W, np.float32)   # (K, F, G)
    res_W = np.asarray(res_W, np.float32)     # (G, F)
    Wk = [cheb_W[0], cheb_W[1], cheb_W[2], res_W.T]
    w4 = np.zeros((4, 128, 128), np.float32)
    for k in range(4):
        for c4 in range(4):
            c2 = c4 % 2
            w4[k, c4 * 32:(c4 + 1) * 32, c2 * 64:(c2 + 1) * 64] = Wk[k]
    b1 = (np.asarray(cheb_b, np.float32) + np.asarray(res_b, np.float32))
    biast = np.tile(b1, 2).reshape(128, 1).astype(np.float32)

    rep = np.zeros((T, DCH * 128), np.float32)
    for p in range(DCH):
        for a in range(4):
            u = 4 * p + a
            rep[u, p * 128 + 32 * a: p * 128 + 32 * a + 32] = 1.0
    p32 = np.zeros((128, 128), np.float32)
    for a in range(4):
        for b_ in range(4):
            p32[a * 32:(a + 1) * 32, b_ * 32:(b_ + 1) * 32] = np.eye(32)

    return {
        "vet": np.ascontiguousarray(np.asarray(Ve, np.float32).T),
        "be": np.ascontiguousarray(np.asarray(be, np.float32)[0]),
        "rep": rep,
        "p32": p32.astype(BF),
        "vst": np.ascontiguousarray(np.asarray(Vs, np.float32).T).astype(BF),
        "bs": np.ascontiguousarray(np.asarray(bs, np.float32)[0]),
        "lt": np.ascontiguousarray(L.T.astype(np.float32)).astype(BF),
        "w4": w4.astype(BF),
        "biast": biast,
        "identb": np.eye(128, dtype=np.float32).astype(BF),
    }


TRACE = False
LAST = {}


def kernel(x, edge_index, edge_weight, Ve, be, Vs, bs, cheb_W, cheb_b, res_W, res_b):
    import ml_dtypes
    from concourse.bass_utils import run_bass_kernel_spmd
    BF = ml_dtypes.bfloat16

    x = np.asarray(x, np.float32)
    shared = _host_prep(x, edge_index, edge_weight, Ve, be, Vs, bs,
                        cheb_W, cheb_b, res_W, res_b)
    nc = _build()
    in_maps = []
    for b in range(B):
        m = dict(shared)
        m["xb"] = np.ascontiguousarray(x[b].reshape(N, D)).astype(BF)
        # xbt: row d' = t*32+f  ->  x[b][n, f, t];  (D, N)
        m["xbt"] = np.ascontiguousarray(
            x[b].transpose(2, 1, 0).reshape(D, N)).astype(BF)
        in_maps.append(m)
    res = run_bass_kernel_spmd(nc, in_maps, list(range(B)), trace=TRACE)
    LAST["res"] = res
    # out (1536, 1024): row = c*128 + a*64 + g, t = 2c+a
    out = np.stack(
        [r["out"].reshape(12, 2, G, N).transpose(3, 2, 0, 1).reshape(N, G, T)
         for r in res.results], axis=0)
    return out
